# revision 17
# baseline (speedup 1.0000x reference)
"""Trainium2 Bass kernel for nn_MiddleOut (gnn_message_passing).

Math (reference):
    out[b,r] = mean_p[ m[b,p] * (my@Wm.T + bias + peer[b,p]@Wp.T + m[b,p]*wm)[r] ]
Collapses to (P = #peers):
    s1[b] = sum_p m[b,p];  s2[b] = sum_p m[b,p]^2
    z[b,l] = sum_p m[b,p] * peer[b,p,l]
    out = s1/P * (my@Wm.T + bias) + (1/P)*(z@Wp.T) + s2/P * wm

Sharding: pure data parallel over batch across 8 cores.

On-device strategy per core (Bc=2048 rows, 16 tiles of 128):
  - peer tile host-permuted to [(b4,p)=128 partitions, j=16, i=2, l=256]
    (batch b_local = (2j+i)*4 + b4), cast to fp8 e4m3 on host (memory-bound
    problem: quarters the dominant stream vs f32).
  - The weighted peer-reduction z runs on the TensorEngine in fp8
    DoubleRowSwInterleave mode: per group-pair j the stationary [128, 2, 128]
    holds the metric diagonal bands of groups 2j/2j+1 pre-interleaved in the
    hw's contiguous fill order (column m of half i at flat 2*(127-m)+i), so
    weight loads stay contiguous (FWL-eligible) while 16 chained matmuls
    PSUM-accumulate psum_z[b_local, l] at 2 fp8 columns/cycle.
  - Bands live in zeroed ping-pong tiles whose band slots are rewritten each
    tile by 4 strided DVE copies; s1/s2 are computed on the host and shipped
    both per-partition (column) and pre-transposed (row) form.
  - my-part needs no runtime transpose: host supplies myT (bf16) chunks used
    as stationary against fixed WmT/P moving chunks -> psum_A = my@Wm.T/P.
  - z is PE-transposed in 2 128-col chunks (f32r), evacuated by ACT copies,
    fed as stationary vs WpT/P moving; plus a K=2 rank-2 matmul
    [s2row;s1row] @ [wm;bias]/P -> psum_B = (z@Wp.T + s2*wm + s1*bias)/P.
  - out = s1 (.) psum_A + psum_B: ACT scales psum_A by s1 (per-partition),
    DVE adds psum_B, batched out-DMA every 4 tiles.
  - All small tensors (metrics, myT, s1/s2 rows) are DMA'd once for the whole
    kernel: 3 descriptors instead of 3 per tile.
"""

import ml_dtypes
import numpy as np

import concourse.bass as bass
import concourse.mybir as mybir
import concourse.tile as tile
from concourse import bacc
from concourse.bass_utils import run_bass_kernel_spmd

F32 = mybir.dt.float32
F32R = mybir.dt.float32r
BF16 = mybir.dt.bfloat16
FP8 = mybir.dt.float8e4

B, P, L, R = 16384, 32, 256, 256
N_CORES = 8
BC = B // N_CORES          # 2048 batches per core
TILE_B = 128               # batches per SBUF tile
NT = BC // TILE_B          # 16 tiles
G = TILE_B // 4            # 32 groups of 4 batches
NJ = G // 2                # 16 group-pairs (DoubleRow does 2 groups/matmul)
OB = 2                     # out-DMA batch (tiles)

SWI = True                 # DoubleRowSwInterleave (contiguous weight reads)

_cache = {}


def build_bass(nt=NT, num_devices=N_CORES):
    bc = nt * TILE_B
    nc = bacc.Bacc(
        "TRN2", target_bir_lowering=False, debug=False, num_devices=num_devices
    )

    x_d = nc.dram_tensor("x", [nt, TILE_B, NJ, 2, L], FP8, kind="ExternalInput")
    # meta packs [mt | s2 | s1] per tile (s1/s2 host-computed)
    meta_d = nc.dram_tensor("meta", [TILE_B, nt, G + 2], F32, kind="ExternalInput")
    s12r_d = nc.dram_tensor("s12r", [2, nt, TILE_B], F32, kind="ExternalInput")
    myt_d = nc.dram_tensor("myt", [TILE_B, nt, 2, TILE_B], BF16, kind="ExternalInput")
    w2_d = nc.dram_tensor("w2", [TILE_B, 2, R], BF16, kind="ExternalInput")   # WmT/P
    wz_d = nc.dram_tensor("wz", [TILE_B, 2, R], F32, kind="ExternalInput")    # WpT/P
    wr_d = nc.dram_tensor("wr", [2, R], F32, kind="ExternalInput")            # [wm;b]/P
    id_d = nc.dram_tensor("ident", [TILE_B, TILE_B], F32, kind="ExternalInput")
    out_d = nc.dram_tensor("out", [nt, TILE_B, R], F32, kind="ExternalOutput")

    perf_mode = (
        mybir.MatmulPerfMode.DoubleRowSwInterleave if SWI
        else mybir.MatmulPerfMode.DoubleRow
    )
    out_bcr = out_d.rearrange("t b r -> b t r")

    with TileCtx(nc) as (tc, ctx):
        singles = ctx.enter_context(tc.tile_pool(name="singles", bufs=1))
        xp = ctx.enter_context(tc.tile_pool(name="xp", bufs=6))
        small = ctx.enter_context(tc.tile_pool(name="small", bufs=6))
        ztp = ctx.enter_context(tc.tile_pool(name="ztp", bufs=4))
        outp = ctx.enter_context(tc.tile_pool(name="outp", bufs=2))
        psz = ctx.enter_context(tc.tile_pool(name="psz", bufs=2, space="PSUM"))
        pst = ctx.enter_context(tc.tile_pool(name="pst", bufs=2, space="PSUM"))
        psa = ctx.enter_context(tc.tile_pool(name="psa", bufs=2, space="PSUM"))
        psb = ctx.enter_context(tc.tile_pool(name="psb", bufs=2, space="PSUM"))

        # whole-kernel loads: meta first (bands gate the first z-chain);
        # weights/myT follow on the scalar queue while x streams on sync
        meta_sb = singles.tile([TILE_B, nt, G + 2], F32)
        nc.scalar.dma_start(out=meta_sb, in_=meta_d[:, :, :])
        w2_sb = singles.tile([TILE_B, 2, R], BF16)
        wz_sb = singles.tile([TILE_B, 2, R], F32R)
        wr_sb = singles.tile([2, R], F32R)
        ident = singles.tile([TILE_B, TILE_B], F32R)
        s12r_sb = singles.tile([2, nt, TILE_B], F32R)
        myt_sb = singles.tile([TILE_B, nt, 2, TILE_B], BF16)

        def load_weights():
            nc.scalar.dma_start(out=w2_sb, in_=w2_d[:, :, :])
            nc.scalar.dma_start(out=wz_sb, in_=wz_d.bitcast(F32R)[:, :, :])
            nc.scalar.dma_start(out=wr_sb, in_=wr_d.bitcast(F32R)[:, :])
            nc.scalar.dma_start(out=ident, in_=id_d.bitcast(F32R)[:, :])
            nc.scalar.dma_start(out=s12r_sb, in_=s12r_d.bitcast(F32R)[:, :, :])
            nc.scalar.dma_start(out=myt_sb, in_=myt_d[:, :, :, :])

        # Ping-pong block-diagonal stationaries for the weighted peer-reduce.
        # SWI storage: column m of half i at flat free 2*(127-m)+i within its
        # 256-block; band elem for (b4, j, ii) thus at 240j + 7ii + 247-2*b4.
        s_tiles = []
        for si in range(3):
            s_i = singles.tile([TILE_B, NJ, 2, TILE_B], FP8, tag=f"s{si}")
            eng = nc.gpsimd if si == 2 else nc.vector
            eng.memset(s_i.bitcast(F32), 0.0)
            s_tiles.append(s_i)

        def stage_fill(t):
            # band elem for (b4, j, ii) at flat 240j + 7ii + 247-2*b4
            s_all = s_tiles[t % 3]
            m_t = meta_sb[:, t, 0:G]
            for b4 in range(4):
                view = s_all[b4 * P:(b4 + 1) * P]
                out_ap = bass.AP(
                    tensor=view.tensor, offset=view.offset + 247 - 2 * b4,
                    ap=[view.ap[0], [240, NJ], [7, 2]],
                )
                nc.vector.tensor_copy(
                    out=out_ap, in_=m_t[b4 * P:(b4 + 1) * P, :],
                )

        def stage_xdma(t):
            x_t = xp.tile([TILE_B, NJ, 2, L], FP8, tag="x_t")
            if t < 2:
                # halves: the first z-chains can start on the first 8 j's
                nc.sync.dma_start(out=x_t[:, 0:NJ // 2], in_=x_d[t, :, 0:NJ // 2])
                nc.sync.dma_start(out=x_t[:, NJ // 2:], in_=x_d[t, :, NJ // 2:])
            else:
                nc.sync.dma_start(out=x_t, in_=x_d[t])
            x_tiles[t] = x_t

        def stage_z(t):
            x_t = x_tiles.pop(t)
            s_all = s_tiles[t % 3]
            psum_z = psz.tile([TILE_B, L], F32, tag="psum_z")
            for j in range(NJ):
                nc.tensor.matmul(
                    out=psum_z,
                    lhsT=s_all[:, j],
                    rhs=x_t[:, j],
                    start=(j == 0),
                    stop=(j == NJ - 1),
                    perf_mode=perf_mode,
                )
            zr = ztp.tile([TILE_B, L], F32R, tag="zr")
            nc.scalar.copy(out=zr, in_=psum_z)
            zrs[t % 3] = zr

        def stage_transp(t):
            zr = zrs[t % 3]
            zts = []
            for c in range(2):
                pt = pst.tile([TILE_B, TILE_B], F32R, tag="pt")
                nc.tensor.transpose(
                    out=pt, in_=zr[:, c * TILE_B:(c + 1) * TILE_B],
                    identity=ident,
                )
                zt = ztp.tile([TILE_B, TILE_B], F32R, tag=f"zt{c}")
                nc.scalar.copy(out=zt, in_=pt)
                zts.append(zt)
            ztss[t % 3] = zts

        def stage_out(t):
            zts = ztss[t % 3]
            psum_b = psb.tile([TILE_B, R], F32, tag="psum_b")
            for c in range(2):
                nc.tensor.matmul(
                    out=psum_b, lhsT=zts[c], rhs=wz_sb[:, c, :],
                    start=(c == 0), stop=False,
                )
            nc.tensor.matmul(
                out=psum_b, lhsT=s12r_sb[:, t, :], rhs=wr_sb,
                start=False, stop=True,
            )
            psum_a = psa.tile([TILE_B, R], F32, tag="psum_a")
            for c in range(2):
                nc.tensor.matmul(
                    out=psum_a, lhsT=myt_sb[:, t, c, :], rhs=w2_sb[:, c, :],
                    start=(c == 0), stop=(c == 1),
                )
            if t % OB == 0:
                out4 = outp.tile([TILE_B, OB, R], F32, tag="out4")
                out4s[0] = out4
            out4 = out4s[0]
            a_sb = small.tile([TILE_B, R], F32, tag="a_sb")
            nc.scalar.activation(
                out=a_sb, in_=psum_a,
                func=mybir.ActivationFunctionType.Copy,
                scale=meta_sb[:, t, G + 1:G + 2],
            )
            nc.vector.tensor_add(out4[:, t % OB, :], a_sb, psum_b)
            if t % OB == OB - 1:
                t0 = t - (OB - 1)
                nc.scalar.dma_start(
                    out=out_bcr[:, t0:t0 + OB, :], in_=out4,
                )

        zrs, ztss, out4s, x_tiles = {}, {}, {}, {}
        # software pipeline: z(t) every iter; transposes batched two tiles
        # per bubble on odd iters (PE transpose-mode switches cost ~1us);
        # out-chains trail two more tiles so every operand is ready at issue
        for t in range(nt + 4):
            if t == 0:
                stage_xdma(0)
                stage_fill(0)
                stage_xdma(1)
            if t < nt:
                if t + 2 < nt:
                    stage_xdma(t + 2)
                if t + 1 < nt:
                    stage_fill(t + 1)
                stage_z(t)
            if t == 0:
                load_weights()
            if t % 2 == 1:
                for u in (t - 2, t - 1):
                    if 0 <= u < nt:
                        stage_transp(u)
                for u in (t - 3, t - 2):
                    if 0 <= u < nt:
                        stage_out(u)

    nc.compile()
    return nc


class TileCtx:
    """with TileCtx(nc) as (tc, ctx): — TileContext plus an ExitStack."""

    def __init__(self, nc):
        from contextlib import ExitStack
        self.tc = tile.TileContext(nc)
        self.ctx = ExitStack()

    def __enter__(self):
        return self.tc.__enter__(), self.ctx.__enter__()

    def __exit__(self, *a):
        self.ctx.__exit__(*a)
        return self.tc.__exit__(*a)


def prep_inputs(my_latent, peer_latents, peer_metrics, W, b):
    """Host-side shard + layout prep (weight packing folds the 1/P mean)."""
    invp = 1.0 / P
    w2 = np.ascontiguousarray(
        (W[:, :L].T * invp).reshape(2, TILE_B, R).transpose(1, 0, 2)
    ).astype(ml_dtypes.bfloat16)                         # [128, 2, R] WmT/P
    wz = np.ascontiguousarray(
        (W[:, L:2 * L].T * invp).reshape(2, TILE_B, R).transpose(1, 0, 2)
    ).astype(np.float32)                                 # [128, 2, R] WpT/P
    wr = np.stack([W[:, 2 * L] * invp, b * invp]).astype(np.float32)  # [2, R]
    ident = np.eye(TILE_B, dtype=np.float32)

    in_maps = []
    for c in range(N_CORES):
        sl = slice(c * BC, (c + 1) * BC)
        # x tile: [(b4,p)=128 partitions, j=16, i=2, l] with b = 8j+4i+b4
        plain = peer_latents[sl].reshape(NT, NJ, 2, 4, P, L)
        xc = np.ascontiguousarray(
            plain.transpose(0, 3, 4, 1, 2, 5).reshape(NT, TILE_B, NJ, 2, L)
        ).astype(ml_dtypes.float8_e4m3)
        mc = peer_metrics[sl]                            # [BC, P]
        s1 = mc.sum(axis=1)                              # [BC]
        s2 = (mc * mc).sum(axis=1)
        # m_t[(b4,p), cidx=2j+ii] = m[4g+b4, p], g = 2j+(1-ii)  (SWI i-flip)
        mt = mc.reshape(NT, G, 4, P).transpose(0, 2, 3, 1)   # [NT, b4, p, g]
        if SWI:
            mt = mt.reshape(NT, 4, P, NJ, 2)[:, :, :, :, ::-1].reshape(
                NT, 4, P, G)
        meta = np.empty((TILE_B, NT, G + 2), dtype=np.float32)
        meta[:, :, 0:G] = mt.reshape(NT, TILE_B, G).transpose(1, 0, 2)
        meta[:, :, G] = s2.reshape(NT, TILE_B).T
        meta[:, :, G + 1] = s1.reshape(NT, TILE_B).T
        s12r = np.stack([s2.reshape(NT, TILE_B), s1.reshape(NT, TILE_B)])
        myt = np.ascontiguousarray(
            my_latent[sl].reshape(NT, TILE_B, 2, TILE_B).transpose(3, 0, 2, 1)
        ).astype(ml_dtypes.bfloat16)                     # [l'=128, NT, 2, b=128]
        in_maps.append({
            "x": xc,
            "meta": meta,
            "s12r": np.ascontiguousarray(s12r).astype(np.float32),
            "myt": myt,
            "w2": w2,
            "wz": wz,
            "wr": wr,
            "ident": ident,
        })
    return in_maps


def run(my_latent, peer_latents, peer_metrics, W, b, trace=False, **kw):
    if "nc" not in _cache:
        _cache["nc"] = build_bass()
    nc = _cache["nc"]
    in_maps = prep_inputs(
        np.asarray(my_latent, dtype=np.float32),
        np.asarray(peer_latents, dtype=np.float32),
        np.asarray(peer_metrics, dtype=np.float32),
        np.asarray(W, dtype=np.float32),
        np.asarray(b, dtype=np.float32),
    )
    res = run_bass_kernel_spmd(
        nc, in_maps, core_ids=list(range(N_CORES)), trace=trace, **kw
    )
    out = np.concatenate(
        [r["out"].reshape(BC, R) for r in res.results], axis=0
    )
    return out, res


def kernel(my_latent, peer_latents, peer_metrics, W, b):
    out, _ = run(my_latent, peer_latents, peer_metrics, W, b)
    return out
